# revision 15
# baseline (speedup 1.0000x reference)
"""CategoryAttention (softmax over heads axis) on 8 Trainium2 cores.

Sharding: B*L = 2*2048 = 4096 query rows split 8 ways (512 rows/core).
Core c handles batch b=c//4, query rows [(c%4)*512, (c%4+1)*512).
The softmax is over the 16 heads, which is fully local to each (q,k)
position, so no cross-core communication is needed. Each core
recomputes K/V projections for its batch (4x redundant).

Per-core pipeline (all layouts transposed so the model dim rides the
SBUF partition axis):
  Q^T = Wq^T.T @ q^T        [1024, 512]
  K^T = Wk^T.T @ k^T        [1024, 2048]
  V   = v^T.T @ Wv^T        [2048, 1024]  (k on partitions)
  per (q-tile 256, k-tile 128):
    e_h^T[k,q] = Kh^T.T @ Qh^T  (16 heads, f32r matmuls)
    p_h = exp(e_h/8)            (ACT, psum->sbuf, bf16)
    den = sum_h p_h             (DVE adds)
    r = 1/den                   (DVE)
    attn = p * r                (DVE, broadcast over h)
    ctx_h^T += Vh.T @ attn_h^T  (PSUM accumulation over all 16 k-tiles)
  out^T = Wo^T.T @ ctx^T + bias
"""

import numpy as np
from contextlib import ExitStack

import concourse.bass as bass
import concourse.tile as tile
from concourse import bacc, mybir
from concourse.bass_utils import run_bass_kernel_spmd

F32 = mybir.dt.float32
F32R = mybir.dt.float32r
BF16 = mybir.dt.bfloat16

N_CORES = 8
P = 128
D = 1024          # d_model
S = D // P        # 8 subtiles of the contraction dim
H = 16            # heads
HD = 64           # head dim
B = 2
L = 2048
LQ = L * B // N_CORES   # 512 query rows per core
LK = L                  # key rows per core (full batch slice)
QT = 256                # q tile
NQT = LQ // QT          # 2
KTS = 128               # k tile
NKT = LK // KTS         # 16
SCALE = 1.0 / np.sqrt(HD)

# dtype knobs
MM_REDUCED = True       # use float32r for the big matmuls
ATT_DT = BF16           # dtype of exp/attn planes and V (AV matmul operands)
import os
STAGE = os.environ.get("KERNEL_STAGE", "full")  # proj | attn | full


MM_DT = F32R if MM_REDUCED else F32


def _mm(ap):
    return ap


def _build(has_bias):
    """Builds the per-core Bass program. has_bias: dict name->bool."""
    nc = bacc.Bacc("TRN2", target_bir_lowering=False, debug=False, num_devices=1)

    def din(name, shape, dt=F32):
        return nc.dram_tensor(name, shape, dt, kind="ExternalInput").ap()

    qT_d = din("qT", (P, S * LQ), MM_DT)
    kT_d = din("kT", (P, S * LK), MM_DT)
    vT_d = din("vT", (P, S * LK), MM_DT)
    wq_d = din("wq", (P, S * D), MM_DT)
    wk_d = din("wk", (P, S * D), MM_DT)
    wv_d = din("wv", (P, S * D), MM_DT)
    wo_d = din("wo", (P, S * D), MM_DT)
    bias_d = {}
    for nm in ("bq", "bk", "bv", "bo"):
        if has_bias[nm]:
            bias_d[nm] = din(nm, (1, D))
    outT_d = nc.dram_tensor("outT", (P, S * LQ), F32, kind="ExternalOutput").ap()

    qT_ap = qT_d.rearrange("p (s q) -> p s q", s=S)
    kT_ap = kT_d.rearrange("p (s k) -> p s k", s=S)
    vT_ap = vT_d.rearrange("p (s k) -> p s k", s=S)
    wq_ap = wq_d.rearrange("p (s o) -> p s o", s=S)
    wk_ap = wk_d.rearrange("p (s o) -> p s o", s=S)
    wv_ap = wv_d.rearrange("p (s o) -> p s o", s=S)
    wo_ap = wo_d.rearrange("p (s o) -> p s o", s=S)
    outT_ap = outT_d.rearrange("p (j q) -> p j q", j=S)

    with tile.TileContext(nc) as tc, ExitStack() as ctx:
        # All streaming (weight halves, input chunks) goes through cpool.
        const_pool = ctx.enter_context(tc.tile_pool(name="const", bufs=1))
        qt_pool = ctx.enter_context(tc.tile_pool(name="QT", bufs=1))
        kt_pool = ctx.enter_context(tc.tile_pool(name="KT", bufs=1))
        v_pool = ctx.enter_context(tc.tile_pool(name="V", bufs=1))

        any_bias = any(has_bias.values())
        ones_t = None
        if any_bias:
            ones_t = const_pool.tile([1, 512], F32, tag="ones")
            nc.vector.memset(ones_t[:], 1.0)
        bias_t = {}
        for nm, d_ap in bias_d.items():
            t = const_pool.tile([1, D], F32, tag=f"bias_{nm}")
            nc.sync.dma_start(t[:], d_ap)
            bias_t[nm] = t

        QT_sb = qt_pool.tile([P, S, LQ], MM_DT)
        KT_sb = kt_pool.tile([P, S, LK], MM_DT)
        V_sb = v_pool.tile([P, NKT, D], ATT_DT)

        with tc.tile_pool(name="chunk", bufs=3) as cpool, \
             tc.tile_pool(name="ppsum", bufs=3, space="PSUM") as ppsum:

            def chunk_tile(src_ap):
                t = cpool.tile([P, S, 512], MM_DT, tag="chunk")
                nc.sync.dma_start(t[:], src_ap)
                return t


            def proj_colmajor(w_halves, in_t, out_sb, bias_name, n_sz, out_col0):
                # out^T[o,n] = sum_i W^T[i,o] * in^T[i,n]; o-tile j on psum
                # partitions. w_halves[wh][:, s, jl*128:...] covers o-tile
                # j = wh*4 + jl.
                for j in range(S):
                    w_t, jl = w_halves[j // 4], j % 4
                    ps = ppsum.tile([P, 512], F32, tag="pp")
                    for s in range(S):
                        nc.tensor.matmul(
                            ps[:, :n_sz],
                            lhsT=_mm(w_t[:, s, jl * P:(jl + 1) * P]),
                            rhs=_mm(in_t[:, s, :]),
                            start=(s == 0),
                            stop=(s == S - 1 and bias_name is None),
                        )
                    if bias_name is not None:
                        nc.tensor.matmul(
                            ps[:, :n_sz],
                            lhsT=bias_t[bias_name][0:1, j * P:(j + 1) * P],
                            rhs=ones_t[0:1, :n_sz],
                            start=False,
                            stop=True,
                        )
                    nc.vector.tensor_copy(out_sb[:, j, out_col0:out_col0 + n_sz],
                                          ps[:, :n_sz])

            # -------- Q projection: Q^T [p, j, q] --------
            qin = chunk_tile(qT_ap)
            wq_h = [chunk_tile(wq_ap[:, :, wh * 512:(wh + 1) * 512])
                    for wh in range(2)]
            proj_colmajor(wq_h, qin, QT_sb,
                          "bq" if has_bias["bq"] else None, LQ, 0)

            # -------- K projection: K^T [p, j, k] --------
            wk_h = [chunk_tile(wk_ap[:, :, wh * 512:(wh + 1) * 512])
                    for wh in range(2)]
            for kn in range(LK // 512):
                kin = chunk_tile(kT_ap[:, :, kn * 512:(kn + 1) * 512])
                proj_colmajor(wk_h, kin, KT_sb,
                              "bk" if has_bias["bk"] else None, 512, kn * 512)

            # -------- V projection: V [p(k), kt, o] ------
            wv_h = [chunk_tile(wv_ap[:, :, wh * 512:(wh + 1) * 512])
                    for wh in range(2)]
            for kn in range(LK // 512):
                vin = chunk_tile(vT_ap[:, :, kn * 512:(kn + 1) * 512])
                for kt4 in range(4):
                    kt = kn * 4 + kt4
                    for on in range(2):
                        ps = ppsum.tile([P, 512], F32, tag="pp")
                        for s in range(S):
                            nc.tensor.matmul(
                                ps[:],
                                lhsT=_mm(vin[:, s, kt4 * P:(kt4 + 1) * P]),
                                rhs=_mm(wv_h[on][:, s, :]),
                                start=(s == 0),
                                stop=(s == S - 1 and not has_bias["bv"]),
                            )
                        if has_bias["bv"]:
                            nc.tensor.matmul(
                                ps[:],
                                lhsT=ones_t[0:1, 0:P],
                                rhs=bias_t["bv"][0:1, on * 512:(on + 1) * 512],
                                start=False,
                                stop=True,
                            )
                        nc.scalar.copy(V_sb[:, kt, on * 512:(on + 1) * 512],
                                       ps[:])

        if STAGE == "proj":
            # debug: dump Q^T projection as the output and stop
            dbg = ctx.enter_context(tc.tile_pool(name="dbg", bufs=2))
            for j in range(S):
                t = dbg.tile([P, LQ], F32, tag="d")
                nc.scalar.copy(t[:], QT_sb[:, j, :].bitcast(F32))
                nc.sync.dma_start(outT_ap[:, j, :], t[:])
        if STAGE != "proj":
            _attention(nc, tc, ctx, has_bias, bias_t, ones_t,
                       QT_sb, KT_sb, V_sb, wo_ap, outT_ap)

    nc.compile()
    return nc


def _attention(nc, tc, ctx, has_bias, bias_t, ones_t,
               QT_sb, KT_sb, V_sb, wo_ap, outT_ap):
    # ---------------- attention + output projection --------------
    # PSUM rules: one matmul target per bank (partition-split base 0/64
    # within a bank is allowed). Energy planes padded to a full bank.
    # AV accumulates in PSUM over KC k-tiles, then DVE drains to SBUF.
    KC = 4
    wo_pool = ctx.enter_context(tc.tile_pool(name="wo", bufs=2))
    attn_pool = ctx.enter_context(tc.tile_pool(name="attn", bufs=KC + 1))
    tree_pool = ctx.enter_context(tc.tile_pool(name="tree", bufs=1))
    den_pool = ctx.enter_context(tc.tile_pool(name="den", bufs=2))
    r_pool = ctx.enter_context(tc.tile_pool(name="r", bufs=2))
    rb_pool = ctx.enter_context(tc.tile_pool(name="rb", bufs=2))
    ctx_pool = ctx.enter_context(tc.tile_pool(name="ctx", bufs=1))
    osb_pool = ctx.enter_context(tc.tile_pool(name="osb", bufs=1))
    e_psum = ctx.enter_context(tc.tile_pool(name="epsum", bufs=1, space="PSUM"))
    av_psum = ctx.enter_context(tc.tile_pool(name="avpsum", bufs=2, space="PSUM"))

    # Wo halves resident for both q tiles
    woh = []
    for j4 in range(2):
        t = wo_pool.tile([P, S, 512], MM_DT, tag="wo")
        nc.sync.dma_start(t[:], wo_ap[:, :, j4 * 512:(j4 + 1) * 512])
        woh.append(t)

    for qt in range(NQT):
        q0 = qt * QT
        ctx_sb = ctx_pool.tile([P, S, QT], MM_DT)
        for c0 in range(0, NKT, KC):
            attn_list = []
            for kt in range(c0, c0 + KC):
                attn_t = attn_pool.tile([P, H, QT], ATT_DT)
                for g in range(4):  # 4 heads per group, one bank per head
                    eps = e_psum.tile([P, 4, 512], F32, tag="e")
                    for hh in range(4):
                        h = g * 4 + hh
                        j2, p0 = h // 2, HD * (h % 2)
                        nc.tensor.matmul(
                            eps[:, hh, 0:QT],
                            lhsT=_mm(KT_sb[p0:p0 + HD, j2, kt * KTS:(kt + 1) * KTS]),
                            rhs=_mm(QT_sb[p0:p0 + HD, j2, q0:q0 + QT]),
                            start=True,
                            stop=True,
                        )
                    nc.scalar.activation(attn_t[:, g * 4:(g + 1) * 4, :],
                                         eps[:, :, 0:QT],
                                         mybir.ActivationFunctionType.Exp,
                                         scale=float(SCALE))
                # den = sum over heads (DVE tree), r = 1/den
                t1 = tree_pool.tile([P, 4, QT], F32)
                nc.vector.tensor_add(t1[:], attn_t[:, 0:4, :], attn_t[:, 4:8, :])
                nc.vector.tensor_add(t1[:], t1[:], attn_t[:, 8:12, :])
                nc.vector.tensor_add(t1[:], t1[:], attn_t[:, 12:16, :])
                nc.vector.tensor_add(t1[:, 0:2, :], t1[:, 0:2, :], t1[:, 2:4, :])
                den = den_pool.tile([P, QT], F32)
                nc.vector.tensor_add(den[:], t1[:, 0, :], t1[:, 1, :])
                r32 = r_pool.tile([P, QT], F32)
                nc.vector.reciprocal(r32[:], den[:])
                if ATT_DT == BF16:
                    rb = rb_pool.tile([P, QT], BF16)
                    nc.scalar.copy(rb[:], r32[:])
                else:
                    rb = r32
                nc.vector.tensor_mul(
                    attn_t[:],
                    attn_t[:],
                    rb[:, None, :].to_broadcast((P, H, QT)),
                )
                attn_list.append(attn_t)
            # AV: 4 avp tiles x (2 pairs x 2 partition-split heads) x KC kts
            for u in range(4):
                avp = av_psum.tile([P, 2, 512], F32, tag="av")
                for ci in range(KC):
                    kt = c0 + ci
                    for hh in range(4):
                        h = 4 * u + hh
                        i, p0 = hh // 2, HD * (hh % 2)
                        nc.tensor.matmul(
                            avp[p0:p0 + HD, i, 0:QT],
                            lhsT=V_sb[:, kt, h * HD:(h + 1) * HD],
                            rhs=attn_list[ci][:, h, :],
                            start=(ci == 0),
                            stop=(ci == KC - 1),
                        )
                if c0 == 0:
                    nc.vector.tensor_copy(ctx_sb[:, 2 * u:2 * u + 2, :],
                                          avp[:, :, 0:QT])
                else:
                    nc.vector.tensor_add(ctx_sb[:, 2 * u:2 * u + 2, :],
                                         ctx_sb[:, 2 * u:2 * u + 2, :],
                                         avp[:, :, 0:QT])
        # output projection
        for j4 in range(2):
            po = e_psum.tile([P, 4, 512], F32, tag="e")
            for jj in range(4):
                for s in range(S):
                    nc.tensor.matmul(
                        po[:, jj, 0:QT],
                        lhsT=_mm(woh[j4][:, s, jj * P:(jj + 1) * P]),
                        rhs=_mm(ctx_sb[:, s, :]),
                        start=(s == 0),
                        stop=(s == S - 1 and not has_bias["bo"]),
                    )
                if has_bias["bo"]:
                    nc.tensor.matmul(
                        po[:, jj, 0:QT],
                        lhsT=bias_t["bo"][0:1, (j4 * 4 + jj) * P:(j4 * 4 + jj + 1) * P],
                        rhs=ones_t[0:1, :QT],
                        start=False,
                        stop=True,
                    )
            osb = osb_pool.tile([P, 4, QT], F32)
            nc.scalar.copy(osb[:], po[:, :, 0:QT])
            nc.sync.dma_start(outT_ap[:, j4 * 4:(j4 + 1) * 4, q0:q0 + QT], osb[:])


_cache = {}


def _get_program(has_bias):
    key = tuple(sorted(has_bias.items()))
    if key not in _cache:
        _cache[key] = _build(has_bias)
    return _cache[key]


def _part_major(x):
    """[S*P, N] -> [P, S*N] partition-major layout for efficient DMA."""
    n = x.shape[1]
    return np.ascontiguousarray(
        x.reshape(S, P, n).transpose(1, 0, 2).reshape(P, S * n))


def prepare_inputs(query, key, value, Wq_w, Wq_b, Wk_w, Wk_b, Wv_w, Wv_b,
                   Wo_w, Wo_b):
    """Host-side sharding/layout. Returns (in_maps, has_bias)."""
    query = np.asarray(query, dtype=np.float32)
    key = np.asarray(key, dtype=np.float32)
    value = np.asarray(value, dtype=np.float32)
    w = {
        "wq": _part_major(np.ascontiguousarray(np.asarray(Wq_w, np.float32).T)),
        "wk": _part_major(np.ascontiguousarray(np.asarray(Wk_w, np.float32).T)),
        "wv": _part_major(np.ascontiguousarray(np.asarray(Wv_w, np.float32).T)),
        "wo": _part_major(np.ascontiguousarray(np.asarray(Wo_w, np.float32).T)),
    }
    biases = {"bq": np.asarray(Wq_b, np.float32), "bk": np.asarray(Wk_b, np.float32),
              "bv": np.asarray(Wv_b, np.float32), "bo": np.asarray(Wo_b, np.float32)}
    has_bias = {nm: bool(np.any(b)) for nm, b in biases.items()}

    kT = [_part_major(np.ascontiguousarray(key[b].T)) for b in range(B)]
    vT = [_part_major(np.ascontiguousarray(value[b].T)) for b in range(B)]

    in_maps = []
    for c in range(N_CORES):
        b, qc = c // (N_CORES // B), c % (N_CORES // B)
        qslice = query[b, qc * LQ:(qc + 1) * LQ, :]
        m = {
            "qT": _part_major(np.ascontiguousarray(qslice.T)),
            "kT": kT[b],
            "vT": vT[b],
            **w,
        }
        for nm, hb in has_bias.items():
            if hb:
                m[nm] = biases[nm].reshape(1, D)
        in_maps.append(m)
    return in_maps, has_bias


def gather_output(results):
    out = np.empty((B, L, D), dtype=np.float32)
    for c in range(N_CORES):
        b, qc = c // (N_CORES // B), c % (N_CORES // B)
        oT = results[c]["outT"].reshape(P, S, LQ).transpose(1, 0, 2).reshape(D, LQ)
        out[b, qc * LQ:(qc + 1) * LQ, :] = oT.T
    return out


def kernel(**inputs) -> np.ndarray:
    in_maps, has_bias = prepare_inputs(**inputs)
    nc = _get_program(has_bias)
    res = run_bass_kernel_spmd(nc, in_maps, list(range(N_CORES)))
    return gather_output(res.results)


if __name__ == "__main__":
    rng = np.random.default_rng(0)
    s = 1.0 / np.sqrt(D)
    inp = {
        "query": rng.standard_normal((B, L, D), dtype=np.float32),
        "key": rng.standard_normal((B, L, D), dtype=np.float32),
        "value": rng.standard_normal((B, L, D), dtype=np.float32),
        "Wq_w": rng.standard_normal((D, D), dtype=np.float32) * s,
        "Wq_b": np.zeros(D, np.float32),
        "Wk_w": rng.standard_normal((D, D), dtype=np.float32) * s,
        "Wk_b": np.zeros(D, np.float32),
        "Wv_w": rng.standard_normal((D, D), dtype=np.float32) * s,
        "Wv_b": np.zeros(D, np.float32),
        "Wo_w": rng.standard_normal((D, D), dtype=np.float32) * s,
        "Wo_b": np.zeros(D, np.float32),
    }
    out = kernel(**inp)
    print("out", out.shape, out.dtype, np.abs(out).mean())


# revision 36
# speedup vs baseline: 15384.1789x; 15384.1789x over previous
"""CategoryAttention (softmax over heads axis) on 8 Trainium2 cores.

Sharding: B*L = 2*2048 = 4096 query rows split 8 ways (512 rows/core).
Core c handles batch b=c//4, query rows [(c%4)*512, (c%4+1)*512).
The softmax is over the 16 heads, which is fully local to each (q,k)
position, so no cross-core communication is needed. Each core
recomputes K/V projections for its batch (4x redundant).

Per-core pipeline (layouts transposed so the model dim rides the SBUF
partition axis; all big matmuls in float32r at full PE rate):
  phase 1: Q^T = Wq^T.T @ q^T;  K^T = Wk^T.T @ k^T;  V = v^T.T @ Wv^T
  phase 2: per k-tile (q = all 512 rows at once):
    e_h^T[k,q] = Kh^T.T @ Qh^T   (16 heads; 2-head psum tiles x3 so
                                  the PE/ACT exp pipeline overlaps)
    p_h = exp(e_h/8)             (ACT, psum->sbuf, bf16)
    den = sum_h p_h              (DVE bf16 tree, f32 final add)
    attn = p * (1/den)           (DVE approx-recip + bcast mult at 2x,
                                  a slice of heads on GPSIMD)
    ctx_h^T += Vh.T @ attn_h^T   (PSUM accum over KC=2 k-tiles,
                                  1 full bank per head-pair, DVE drain)
  phase 3: out^T = Wo^T.T @ ctx^T + bias
"""

import numpy as np
from contextlib import ExitStack

import concourse.bass as bass
import concourse.tile as tile
from concourse import bacc, mybir
from concourse.bass_utils import run_bass_kernel_spmd

F32 = mybir.dt.float32
F32R = mybir.dt.float32r
BF16 = mybir.dt.bfloat16

N_CORES = 8
P = 128
D = 1024          # d_model
S = D // P        # 8 subtiles of the contraction dim
H = 16            # heads
HD = 64           # head dim
B = 2
L = 2048
LQ = L * B // N_CORES   # 512 query rows per core
LK = L                  # key rows per core (full batch slice)
KTS = 128               # k tile
NKT = LK // KTS         # 16
KC = 2                  # k-tiles per AV psum accumulation chunk
GP_HEADS = 0            # heads normalized on GPSIMD instead of DVE
SCALE = 1.0 / np.sqrt(HD)

MM_REDUCED = True       # float32r matmuls
ATT_DT = BF16           # exp/attn planes and V dtype
MM_DT = F32R if MM_REDUCED else F32

import os
BENCH_LOOP = int(os.environ.get("BENCH_LOOP", "1"))


def _build(has_bias):
    nc = bacc.Bacc("TRN2", target_bir_lowering=False, debug=False, num_devices=1)

    def din(name, shape, dt=F32):
        return nc.dram_tensor(name, shape, dt, kind="ExternalInput").ap()

    qT_d = din("qT", (P, S * LQ), MM_DT)
    kT_d = din("kT", (P, S * LK), MM_DT)
    vT_d = din("vT", (P, S * LK), MM_DT)
    wq_d = din("wq", (P, S * D), MM_DT)
    wk_d = din("wk", (P, S * D), MM_DT)
    wv_d = din("wv", (P, S * D), MM_DT)
    wo_d = din("wo", (P, S * D), MM_DT)
    bias_d = {}
    for nm in ("bq", "bk", "bv", "bo"):
        if has_bias[nm]:
            bias_d[nm] = din(nm, (1, D))
    outT_d = nc.dram_tensor("outT", (P, S * LQ), F32, kind="ExternalOutput").ap()

    qT_ap = qT_d.rearrange("p (s q) -> p s q", s=S)
    kT_ap = kT_d.rearrange("p (c s k) -> p c s k", c=4, s=S)
    vT_ap = vT_d.rearrange("p (c s k) -> p c s k", c=4, s=S)
    wq_ap = wq_d.rearrange("p (h s o) -> p h s o", h=2, s=S)
    wk_ap = wk_d.rearrange("p (h s o) -> p h s o", h=2, s=S)
    wv_ap = wv_d.rearrange("p (h s o) -> p h s o", h=2, s=S)
    wo_ap = wo_d.rearrange("p (h s o) -> p h s o", h=2, s=S)
    outT_ap = outT_d.rearrange("p (j q) -> p j q", j=S)

    with tile.TileContext(nc) as tc, ExitStack() as ctx:
        if BENCH_LOOP > 1:
            ctx.enter_context(tc.For_i(0, BENCH_LOOP, 1))

        const_pool = ctx.enter_context(tc.tile_pool(name="const", bufs=1))
        qt_pool = ctx.enter_context(tc.tile_pool(name="QT", bufs=1))
        kt_pool = ctx.enter_context(tc.tile_pool(name="KT", bufs=1))
        v_pool = ctx.enter_context(tc.tile_pool(name="V", bufs=1))

        any_bias = any(has_bias.values())
        ones_t = None
        if any_bias:
            ones_t = const_pool.tile([1, 512], F32, tag="ones")
            nc.vector.memset(ones_t[:], 1.0)
        bias_t = {}
        for nm, d_ap in bias_d.items():
            t = const_pool.tile([1, D], F32, tag=f"bias_{nm}")
            nc.sync.dma_start(t[:], d_ap)
            bias_t[nm] = t

        QT_sb = qt_pool.tile([P, S, LQ], MM_DT)
        KT_sb = kt_pool.tile([P, S, LK], MM_DT)
        V_sb = v_pool.tile([P, NKT, D], ATT_DT)

        def bias_mm(ps_t, bias_name, o0, n_sz, o_on_partitions):
            if o_on_partitions:
                nc.tensor.matmul(ps_t, lhsT=bias_t[bias_name][0:1, o0:o0 + P],
                                 rhs=ones_t[0:1, :n_sz], start=False, stop=True)
            else:
                nc.tensor.matmul(ps_t, lhsT=ones_t[0:1, 0:P],
                                 rhs=bias_t[bias_name][0:1, o0:o0 + n_sz],
                                 start=False, stop=True)

        # ---------------- phase 1: Q, K, V projections ----------------
        with tc.tile_pool(name="stream", bufs=5) as spool, \
             tc.tile_pool(name="ppsum", bufs=2, space="PSUM") as ppsum:

            def stream_tile(src_ap):
                t = spool.tile([P, S, 512], MM_DT, tag="w")
                nc.sync.dma_start(t[:], src_ap)
                return t

            def proj_cols(w_halves, in_t, out_view, bias_name, n_sz):
                # out^T[o,n] = sum_i W^T[i,o]*in^T[i,n]; 4 o-tiles per psum
                # tile (one bank per matmul target), one ACT copy out.
                for j4 in range(2):
                    ps = ppsum.tile([P, 4, 512], F32, tag="pp")
                    for jl in range(4):
                        j = j4 * 4 + jl
                        w_t = w_halves[j // 4]
                        for s in range(S):
                            nc.tensor.matmul(
                                ps[:, jl, :n_sz],
                                lhsT=w_t[:, s, jl * P:(jl + 1) * P],
                                rhs=in_t[:, s, :n_sz],
                                start=(s == 0),
                                stop=(s == S - 1 and bias_name is None),
                            )
                        if bias_name is not None:
                            bias_mm(ps[:, jl, :n_sz], bias_name, j * P, n_sz,
                                    True)
                    nc.scalar.copy(out_view[:, j4 * 4:(j4 + 1) * 4, :n_sz],
                                   ps[:, :, :n_sz])

            qin = stream_tile(qT_ap)
            wq_h = [stream_tile(wq_ap[:, wh]) for wh in range(2)]
            proj_cols(wq_h, qin, QT_sb, "bq" if has_bias["bq"] else None, LQ)

            wk_h = [stream_tile(wk_ap[:, wh]) for wh in range(2)]
            for kn in range(4):
                kin = stream_tile(kT_ap[:, kn])
                proj_cols(wk_h, kin, KT_sb[:, :, kn * 512:(kn + 1) * 512],
                          "bk" if has_bias["bk"] else None, 512)

            wv_h = [stream_tile(wv_ap[:, wh]) for wh in range(2)]
            for kn in range(4):
                vin = stream_tile(vT_ap[:, kn])
                for kt4 in range(4):
                    kt = kn * 4 + kt4
                    ps = ppsum.tile([P, 4, 512], F32, tag="pp")
                    for t in range(2):  # o halves; 2 targets used of 4
                        for s in range(S):
                            nc.tensor.matmul(
                                ps[:, t, :],
                                lhsT=vin[:, s, kt4 * P:(kt4 + 1) * P],
                                rhs=wv_h[t][:, s, :],
                                start=(s == 0),
                                stop=(s == S - 1 and not has_bias["bv"]),
                            )
                        if has_bias["bv"]:
                            bias_mm(ps[:, t, :], "bv", t * 512, 512, False)
                    nc.scalar.copy(V_sb[:, kt, :],
                                   ps[:, 0:2, :].rearrange("p a b -> p (a b)"))

        # ---------------- phase 2: attention (q = 512) ----------------
        attn_pool = ctx.enter_context(tc.tile_pool(name="attn", bufs=3))
        tree_pool = ctx.enter_context(tc.tile_pool(name="tree", bufs=1))
        den_pool = ctx.enter_context(tc.tile_pool(name="den", bufs=1))
        r_pool = ctx.enter_context(tc.tile_pool(name="r", bufs=1))
        rb_pool = ctx.enter_context(tc.tile_pool(name="rb", bufs=1))
        ctx_pool = ctx.enter_context(tc.tile_pool(name="ctx", bufs=1))
        osb_pool = ctx.enter_context(tc.tile_pool(name="osb", bufs=1))
        wo_pool = ctx.enter_context(tc.tile_pool(name="wo", bufs=1))
        e_psum = ctx.enter_context(tc.tile_pool(name="epsum", bufs=3, space="PSUM"))
        av_psum = ctx.enter_context(tc.tile_pool(name="avpsum", bufs=1, space="PSUM"))

        ctx_sb = ctx_pool.tile([P, S, LQ], MM_DT)

        def softmax_kt(kt):
            """Energy (16 heads) -> exp -> den -> normalized attn tile."""
            attn_t = attn_pool.tile([P, H, LQ], ATT_DT, tag="attn")
            for g in range(8):  # 2 heads per psum tile, one bank per head
                eps = e_psum.tile([P, 2, LQ], F32, tag="e")
                for hh in range(2):
                    h = g * 2 + hh
                    j2, p0 = h // 2, HD * (h % 2)
                    nc.tensor.matmul(
                        eps[:, hh, :],
                        lhsT=KT_sb[p0:p0 + HD, j2, kt * KTS:(kt + 1) * KTS],
                        rhs=QT_sb[p0:p0 + HD, j2, :],
                        start=True,
                        stop=True,
                    )
                nc.scalar.activation(attn_t[:, g * 2:(g + 1) * 2, :], eps[:],
                                     mybir.ActivationFunctionType.Exp,
                                     scale=float(SCALE))
            # den = sum over heads (bf16 tree at DVE 2x; final add f32)
            t1 = tree_pool.tile([P, 4, LQ], ATT_DT)
            with nc.allow_low_precision(reason="bf16 head-sum tree"):
                nc.vector.tensor_add(t1[:], attn_t[:, 0:4, :], attn_t[:, 4:8, :])
                nc.vector.tensor_add(t1[:], t1[:], attn_t[:, 8:12, :])
                nc.vector.tensor_add(t1[:], t1[:], attn_t[:, 12:16, :])
                nc.vector.tensor_add(t1[:, 0:2, :], t1[:, 0:2, :], t1[:, 2:4, :])
            den = den_pool.tile([P, LQ], F32)
            nc.vector.tensor_add(den[:], t1[:, 0, :], t1[:, 1, :])
            r32 = r_pool.tile([P, LQ], F32)
            nc.vector.reciprocal_approx_fast(r32[:], den[:])
            rb = rb_pool.tile([P, LQ], ATT_DT)
            nc.gpsimd.tensor_copy(rb[:], r32[:])
            nd = H - GP_HEADS
            nc.vector.tensor_mul(
                attn_t[:, 0:nd, :], attn_t[:, 0:nd, :],
                rb[:, None, :].to_broadcast((P, nd, LQ)))
            if GP_HEADS:
                nc.gpsimd.tensor_mul(
                    attn_t[:, nd:H, :], attn_t[:, nd:H, :],
                    rb[:, None, :].to_broadcast((P, GP_HEADS, LQ)))
            return attn_t

        def av_group(u, c0, attn_list, first):
            """One avp tile: pairs (2u, 2u+1), full q, over KC k-tiles."""
            avp = av_psum.tile([P, 2, LQ], F32, tag="av")
            for ci in range(KC):
                kt = c0 + ci
                for hh in range(4):
                    h = 4 * u + hh
                    i, p0 = hh // 2, HD * (hh % 2)
                    nc.tensor.matmul(
                        avp[p0:p0 + HD, i, :],
                        lhsT=V_sb[:, kt, h * HD:(h + 1) * HD],
                        rhs=attn_list[ci][:, h, :],
                        start=(ci == 0),
                        stop=(ci == KC - 1),
                    )
            if first:
                nc.vector.tensor_copy(ctx_sb[:, 2 * u:2 * u + 2, :],
                                      avp[:, :, :])
            else:
                nc.vector.tensor_add(ctx_sb[:, 2 * u:2 * u + 2, :],
                                     ctx_sb[:, 2 * u:2 * u + 2, :],
                                     avp[:, :, :])

        prev = None  # (c0, attn_list)
        for ch in range(NKT // KC):
            c0 = ch * KC
            cur = []
            for ci in range(KC):
                cur.append(softmax_kt(c0 + ci))
                if prev is not None:
                    for u in (2 * ci, 2 * ci + 1):
                        av_group(u, prev[0], prev[1], prev[0] == 0)
            prev = (c0, cur)
        for u in range(4):
            av_group(u, prev[0], prev[1], False)

        # ---------------- phase 3: output projection ----------------
        for j4 in range(2):
            woh = wo_pool.tile([P, S, 512], MM_DT, tag="wo")
            nc.sync.dma_start(woh[:], wo_ap[:, j4])
            for j2 in range(2):
                po = e_psum.tile([P, 2, LQ], F32, tag="e")
                for jj in range(2):
                    j = j4 * 4 + j2 * 2 + jj
                    jl = j2 * 2 + jj
                    for s in range(S):
                        nc.tensor.matmul(
                            po[:, jj, :],
                            lhsT=woh[:, s, jl * P:(jl + 1) * P],
                            rhs=ctx_sb[:, s, :],
                            start=(s == 0),
                            stop=(s == S - 1 and not has_bias["bo"]),
                        )
                    if has_bias["bo"]:
                        bias_mm(po[:, jj, :], "bo", j * P, LQ, True)
                osb = osb_pool.tile([P, 2, LQ], F32)
                nc.scalar.copy(osb[:], po[:])
                j0 = j4 * 4 + j2 * 2
                nc.sync.dma_start(outT_ap[:, j0:j0 + 2, :], osb[:])

    nc.compile()
    return nc


_cache = {}


def _get_program(has_bias):
    key = (BENCH_LOOP, tuple(sorted(has_bias.items())))
    if key not in _cache:
        _cache[key] = _build(has_bias)
    return _cache[key]


def _part_major(x):
    n = x.shape[1]
    return np.ascontiguousarray(
        x.reshape(S, P, n).transpose(1, 0, 2).reshape(P, S * n))


def _chunked(x, width=512):
    """[D, N] -> [P, N//width, S, width] per-chunk contiguous layout."""
    n = x.shape[1]
    nch = n // width
    y = x.reshape(S, P, nch, width).transpose(1, 2, 0, 3)
    return np.ascontiguousarray(y.reshape(P, nch * S * width))


def prepare_inputs(query, key, value, Wq_w, Wq_b, Wk_w, Wk_b, Wv_w, Wv_b,
                   Wo_w, Wo_b):
    query = np.asarray(query, dtype=np.float32)
    key = np.asarray(key, dtype=np.float32)
    value = np.asarray(value, dtype=np.float32)
    w = {
        "wq": _chunked(np.ascontiguousarray(np.asarray(Wq_w, np.float32).T)),
        "wk": _chunked(np.ascontiguousarray(np.asarray(Wk_w, np.float32).T)),
        "wv": _chunked(np.ascontiguousarray(np.asarray(Wv_w, np.float32).T)),
        "wo": _chunked(np.ascontiguousarray(np.asarray(Wo_w, np.float32).T)),
    }
    biases = {"bq": np.asarray(Wq_b, np.float32), "bk": np.asarray(Wk_b, np.float32),
              "bv": np.asarray(Wv_b, np.float32), "bo": np.asarray(Wo_b, np.float32)}
    has_bias = {nm: bool(np.any(b)) for nm, b in biases.items()}

    kT = [_chunked(np.ascontiguousarray(key[b].T)) for b in range(B)]
    vT = [_chunked(np.ascontiguousarray(value[b].T)) for b in range(B)]

    in_maps = []
    for c in range(N_CORES):
        b, qc = c // (N_CORES // B), c % (N_CORES // B)
        qslice = query[b, qc * LQ:(qc + 1) * LQ, :]
        m = {
            "qT": _part_major(np.ascontiguousarray(qslice.T)),
            "kT": kT[b],
            "vT": vT[b],
            **w,
        }
        for nm, hb in has_bias.items():
            if hb:
                m[nm] = biases[nm].reshape(1, D)
        in_maps.append(m)
    return in_maps, has_bias


def gather_output(results):
    out = np.empty((B, L, D), dtype=np.float32)
    for c in range(N_CORES):
        b, qc = c // (N_CORES // B), c % (N_CORES // B)
        oT = results[c]["outT"].reshape(P, S, LQ).transpose(1, 0, 2).reshape(D, LQ)
        out[b, qc * LQ:(qc + 1) * LQ, :] = oT.T
    return out


def kernel(**inputs) -> np.ndarray:
    in_maps, has_bias = prepare_inputs(**inputs)
    nc = _get_program(has_bias)
    res = run_bass_kernel_spmd(nc, in_maps, list(range(N_CORES)))
    return gather_output(res.results)
